# revision 18
# baseline (speedup 1.0000x reference)
"""CLIP-style contrastive (NT-Xent) loss on 8 Trainium2 NeuronCores.

Strategy (data-parallel, per sharding hint):
  - Shard the batch (4096) across 8 cores: 512 rows of x_image/x_text each.
  - Each core projects its shard through both towers in TRANSPOSED
    activation layout ([feat_partitions, batch_free]) so every Linear uses
    the stored weight directly as the stationary lhsT (out = lhsT.T @ rhs).
  - ALL per-tower operands (xT chunks, We, Wp1, Wp2) are packed host-side
    into ONE [128, 9472] bf16 DRAM tensor in the exact SBUF image, so the
    whole tower loads with a single DMA instruction (descriptor generation
    on the issuing queue costs ~600ns per dma_start, so the baseline's 73
    small DMAs burned ~44us of sync-queue time).
  - A 4-byte dummy AllGather is triggered at t=0 so the one-time CC
    bootstrap barrier (~40us) overlaps the projection phase instead of
    gating the first real AllGather.
  - L2-normalize the 128-dim projections on-device: ssq via ones-matmul
    broadcast, Sqrt on ScalarE, fast reciprocal on the DVE (avoids the
    baseline's Ln/Exp activation-table thrash: each table load is 1.3us).
  - AllGather the bf16 normalized projections per modality (img AG
    overlaps the txt tower; txt AG overlaps the img-column sim work).
  - Each core computes its 1024 rows of the global 8192x8192 similarity
    matrix in [128, 2048] PSUM chunks (bf16 matmuls), applies exp(sim/t)
    on ScalarE with fused per-row accumulation (accum_out), giving row
    sums T_r including the self-similarity diagonal.
  - Device returns, per row: T_r, diag_r, pos_r.  Host finishes in fp64:
        T'_r   = T_r - exp(diag_r/t) + exp(pos_r/t)
        loss_r = log(T'_r) - pos_r/t
"""

import os

# Small (256KB/rank) intra-chip AllGathers sit on the critical path; the
# mesh algorithm beats RDH for them (measured 171us -> 164us end-to-end).
os.environ.setdefault("NEURON_RT_DBG_RDH_CC", "1")

import numpy as np
import ml_dtypes

import concourse.bacc as bacc
import concourse.bass as bass
import concourse.mybir as mybir
import concourse.tile as tile
from concourse.bass_utils import run_bass_kernel_spmd
from concourse.dve_ops import RECIPROCAL_APPROX_FAST, RECIP_APPROX_FAST_CONSTS

NCORES = 8
B, DIN, DE, DH, DP = 4096, 1024, 512, 256, 128
S = B // NCORES            # 512: per-core batch shard
ROWS = 2 * S               # 1024 sim rows owned per core (z1 + z2 shard)
N = 2 * B                  # 8192 global rows
TEMP = 0.07
INV_T = 1.0 / TEMP

F32 = mybir.dt.float32
BF16 = mybir.dt.bfloat16
SIM_DT = BF16              # dtype of the similarity matmul operands
PROJ_DT = BF16             # dtype of projection matmul operands
NP_PROJ = ml_dtypes.bfloat16 if PROJ_DT == BF16 else np.float32

# packed tower tensor column offsets (all in PROJ_DT elements)
OXT = 0                    # xT chunks:   8 x [128, 512]
OWE = OXT + (DIN // 128) * S       # We chunks:  8 x [128, 512]
OW1 = OWE + (DIN // 128) * DE      # Wp1 chunks: 4 x [128, 256]
OW2 = OW1 + (DE // 128) * DH       # Wp2 chunks: 2 x [128, 128]
TWC = OW2 + (DH // 128) * DP       # 9472 total columns

# bias_all fp32 [128, 14]: beT(4)|bp1T(2)|bp2T(1) img then txt
BIAS_OFF = {"img": 0, "txt": 7}

# device output layout: [128, 20] = T(8) | pos(4) | diag_img(4) | diag_txt(4)
OUT_COLS = 20

_CACHE: dict = {}


def _build():
    nc = bacc.Bacc("TRN2", target_bir_lowering=False, debug=False,
                   num_devices=NCORES)

    t_in = {
        "tw_img": nc.dram_tensor("tw_img", [128, TWC], PROJ_DT,
                                 kind="ExternalInput"),
        "tw_txt": nc.dram_tensor("tw_txt", [128, TWC], PROJ_DT,
                                 kind="ExternalInput"),
        "bias_all": nc.dram_tensor("bias_all", [128, 14], F32,
                                   kind="ExternalInput"),
    }
    out_t = nc.dram_tensor("parts", [128, OUT_COLS], F32,
                           kind="ExternalOutput")

    with tile.TileContext(nc) as tc:
        _emit(nc, tc, t_in, out_t)
    nc.compile()
    return nc


def _project_normalize(nc, pps, psb, apool, tw, ball, m, ones128):
    """Emit one tower: projections (bf16 matmuls) + fp32 normalize.

    Returns (zn fp32 [128,512], znb SIM_DT [128,512])."""
    add = mybir.AluOpType.add
    mx = mybir.AluOpType.max
    bo = BIAS_OFF[m]

    h = psb.tile([128, (DE // 128) * S], PROJ_DT, tag="h")
    for mm in range(DE // 128):
        ph = pps.tile([128, S], F32, tag="simps")
        for k in range(DIN // 128):
            nc.tensor.matmul(
                ph[:],
                tw[:, OWE + k * DE + 128 * mm: OWE + k * DE + 128 * (mm + 1)],
                tw[:, OXT + k * S: OXT + (k + 1) * S],
                start=(k == 0), stop=(k == DIN // 128 - 1))
        nc.vector.tensor_scalar(
            out=h[:, mm * S:(mm + 1) * S], in0=ph[:],
            scalar1=ball[:, bo + mm: bo + mm + 1], scalar2=None, op0=add)
    g = psb.tile([128, (DH // 128) * S], PROJ_DT, tag="g")
    for mm in range(DH // 128):
        pg = pps.tile([128, S], F32, tag="simps")
        for k in range(DE // 128):
            nc.tensor.matmul(
                pg[:],
                tw[:, OW1 + k * DH + 128 * mm: OW1 + k * DH + 128 * (mm + 1)],
                h[:, k * S:(k + 1) * S],
                start=(k == 0), stop=(k == DE // 128 - 1))
        nc.vector.tensor_scalar(
            out=g[:, mm * S:(mm + 1) * S], in0=pg[:],
            scalar1=ball[:, bo + 4 + mm: bo + 5 + mm], scalar2=0.0,
            op0=add, op1=mx)
    pz = pps.tile([128, S], F32, tag="simps")
    for k in range(DH // 128):
        nc.tensor.matmul(pz[:], tw[:, OW2 + k * DP: OW2 + (k + 1) * DP],
                         g[:, k * S:(k + 1) * S],
                         start=(k == 0), stop=(k == DH // 128 - 1))
    z = psb.tile([128, S], F32, tag=f"z_{m}")
    nc.vector.tensor_scalar(out=z[:], in0=pz[:],
                            scalar1=ball[:, bo + 6: bo + 7],
                            scalar2=None, op0=add)

    # normalize columns (rows of z): inv = 1/sqrt(colsum(z^2)), broadcast
    # to all 128 partitions by the ones128 matmul so the reciprocal runs
    # full-width on the DVE.
    sq = psb.tile([128, S], F32, tag="sq")
    nc.vector.tensor_mul(sq[:], z[:], z[:])
    pssqb = pps.tile([128, S], F32, tag="simps")
    nc.tensor.matmul(pssqb[:], ones128[:], sq[:], start=True, stop=True)
    nrm = psb.tile([128, S], F32, tag="nrm")
    nc.scalar.activation(nrm[:], pssqb[:], mybir.ActivationFunctionType.Sqrt)
    inv = psb.tile([128, S], F32, tag="inv")
    nc.vector._custom_dve(
        RECIPROCAL_APPROX_FAST, out=inv[:], in0=nrm[:],
        s0=RECIP_APPROX_FAST_CONSTS["s0"], s1=RECIP_APPROX_FAST_CONSTS["s1"],
        imm2=RECIP_APPROX_FAST_CONSTS["imm2"])
    zn = apool.tile([128, S], F32, name=f"zn_{m}")
    nc.vector.tensor_mul(zn[:], z[:], inv[:])
    znb = apool.tile([128, S], SIM_DT, name=f"znb_{m}")
    nc.vector.tensor_copy(znb[:], zn[:])
    return zn, znb


def _emit(nc, tc, t_in, out_t):
    Exp = mybir.ActivationFunctionType.Exp
    add = mybir.AluOpType.add

    NCHUNK = 2048                  # columns per PSUM super-chunk (4 banks)
    NTT = N // NCHUNK              # 4
    NRC = ROWS // 128              # 8 row chunks

    with tc.tile_pool(name="const", bufs=1) as cpool, \
         tc.tile_pool(name="wpool", bufs=1) as wpool, \
         tc.tile_pool(name="actpool", bufs=1) as apool, \
         tc.tile_pool(name="projsb", bufs=2) as psb, \
         tc.tile_pool(name="psum", bufs=2, space="PSUM") as pps, \
         tc.tile_pool(name="escp", bufs=2) as escp, \
         tc.tile_pool(name="dram", bufs=1, space="DRAM") as dram:

        # ---- bulk input DMAs: one instruction per packed tensor ----
        tw = {"img": wpool.tile([128, TWC], PROJ_DT, name="tw_img"),
              "txt": wpool.tile([128, TWC], PROJ_DT, name="tw_txt")}
        nc.sync.dma_start(out=tw["img"][:], in_=t_in["tw_img"][:, :])
        nc.scalar.dma_start(out=tw["txt"][:], in_=t_in["tw_txt"][:, :])
        ball = wpool.tile([128, 14], F32, name="ball")
        nc.sync.dma_start(out=ball[:], in_=t_in["bias_all"][:, :])

        ones128 = cpool.tile([128, 128], F32)
        nc.vector.memset(ones128[:], 1.0)

        zn, znb, cc_out = {}, {}, {}
        zf = {"img": apool.tile([128, B], SIM_DT, name="zf_img"),
              "txt": apool.tile([128, B], SIM_DT, name="zf_txt")}
        for m in ("img", "txt"):
            zn[m], znb[m] = _project_normalize(
                nc, pps, psb, apool, tw[m], ball, m, ones128)
            # AllGather this modality right away (img AG overlaps txt tower,
            # txt AG overlaps the img-column sim supersteps).
            cc_in = dram.tile([128, S], SIM_DT, name=f"cc_in_{m}")
            nc.scalar.dma_start(out=cc_in[:, :], in_=znb[m][:])
            cc_o = dram.tile([128 * NCORES, S], SIM_DT, name=f"cc_out_{m}",
                             addr_space="Shared")
            nc.gpsimd.collective_compute(
                "AllGather", mybir.AluOpType.bypass,
                replica_groups=[list(range(NCORES))],
                ins=[cc_in[:]], outs=[cc_o[:]])
            cc_out[m] = cc_o
            # one rearranged DMA gathers all 8 rank slabs into SBUF
            nc.sync.dma_start(
                out=zf[m][:].rearrange("p (j c) -> p j c", j=NCORES),
                in_=cc_o[:].rearrange("(j p) c -> p j c", p=128))

        # Warm the Exp activation table on ScalarE now (pre-barrier dead
        # window) so the first sim activation skips the 1.3us table load.
        warm = psb.tile([1, 16], F32, tag="rowsb")
        nc.scalar.activation(warm[:], ones128[0:1, 0:16], Exp)

        # pos / self-diag rows ([1, 512] each) -> [128, 12] via DRAM bounce.
        # Products use znb (the bf16 values the sim matmul actually sees) so
        # the host's diag/pos correction cancels the in-matrix terms exactly.
        rows_d = dram.tile([3, S], F32)
        for r, (a, b) in enumerate((("img", "txt"), ("img", "img"),
                                    ("txt", "txt"))):
            prod = psb.tile([128, S], F32, tag="sq")
            nc.vector.tensor_mul(prod[:], znb[a][:], znb[b][:])
            pr = pps.tile([1, S], F32, tag="simps")
            nc.tensor.matmul(pr[:], ones128[:, 0:1], prod[:], start=True,
                             stop=True)
            row_sb = psb.tile([1, S], F32, tag="rowsb")
            nc.vector.tensor_copy(row_sb[:], pr[:])
            nc.scalar.dma_start(out=rows_d[r:r + 1, :], in_=row_sb[:])

        pdT = apool.tile([128, 12], F32)   # pos | diag_img | diag_txt
        nc.scalar.dma_start(
            out=pdT[:],
            in_=rows_d[:, :].rearrange("r (c p) -> p (r c)", p=128))

        # ---- main loop: sim rows + exp + fused row sums ----
        # image columns (ready after AG1) run before text columns.
        stats = apool.tile([128, NRC * NTT], F32)
        for tt in range(NTT):
            src = zf["img"] if tt < NTT // 2 else zf["txt"]
            coff = (tt % (NTT // 2)) * NCHUNK
            for rc in range(NRC):
                if rc < 4:
                    lhs = znb["img"][:, 128 * rc:128 * (rc + 1)]
                else:
                    lhs = znb["txt"][:, 128 * (rc - 4):128 * (rc - 3)]
                ps = pps.tile([128, NCHUNK], F32, tag="simps")
                for q in range(NCHUNK // 512):
                    nc.tensor.matmul(
                        ps[:, 512 * q:512 * (q + 1)], lhs,
                        src[:, coff + 512 * q: coff + 512 * (q + 1)],
                        start=True, stop=True)
                esc = escp.tile([128, NCHUNK], SIM_DT, tag="esc")
                nc.scalar.activation(
                    esc[:], ps[:], Exp, scale=INV_T,
                    accum_out=stats[:, NTT * rc + tt: NTT * rc + tt + 1])

        # ---- gather outputs: T (8) | pos(4) | diag1(4) | diag2(4) ----
        outv = apool.tile([128, OUT_COLS], F32)
        nc.vector.tensor_reduce(
            out=outv[:, 0:NRC],
            in_=stats[:].rearrange("p (r t) -> p r t", t=NTT),
            axis=mybir.AxisListType.X, op=add)
        nc.vector.tensor_copy(outv[:, NRC:NRC + 12], pdT[:])
        nc.sync.dma_start(out=out_t[:, :], in_=outv[:])


def _chunkT(a):
    """[R, C] (R = 128*k) -> [128, k*C] with chunk k at cols [k*C, (k+1)*C)."""
    r, c = a.shape
    return a.reshape(r // 128, 128, c).transpose(1, 0, 2).reshape(128, -1)


def _prep_in_maps(inputs):
    wpart, biases = {}, {}
    for m in ("img", "txt"):
        wpart[m] = np.concatenate(
            [_chunkT(np.asarray(inputs[f"We_{m}"], np.float32)),
             _chunkT(np.asarray(inputs[f"Wp1_{m}"], np.float32)),
             _chunkT(np.asarray(inputs[f"Wp2_{m}"], np.float32))],
            axis=1).astype(NP_PROJ)
        biases[m] = np.concatenate(
            [np.asarray(inputs[f"be_{m}"], np.float32).reshape(DE // 128, 128).T,
             np.asarray(inputs[f"bp1_{m}"], np.float32).reshape(DH // 128, 128).T,
             np.asarray(inputs[f"bp2_{m}"], np.float32).reshape(DP // 128, 128).T],
            axis=1)
    bias_all = np.ascontiguousarray(
        np.concatenate([biases["img"], biases["txt"]], axis=1))
    x = {"img": np.asarray(inputs["x_image"], np.float32),
         "txt": np.asarray(inputs["x_text"], np.float32)}
    in_maps = []
    for c in range(NCORES):
        mp = {"bias_all": bias_all}
        for m in ("img", "txt"):
            xt = _chunkT(x[m][c * S:(c + 1) * S].T).astype(NP_PROJ)
            mp[f"tw_{m}"] = np.ascontiguousarray(
                np.concatenate([xt, wpart[m]], axis=1))
        in_maps.append(mp)
    return in_maps


def _finish_host(results):
    """Host-side fp64 finish: combine per-core T/pos/diag into the loss."""
    total = 0.0
    t = TEMP
    for c in range(NCORES):
        p = np.asarray(results[c]["parts"], np.float64)
        T = p[:, 0:8]           # [128, rc]
        pos = p[:, 8:12]        # [128, k]  (k = batch chunk within shard)
        d1 = p[:, 12:16]
        d2 = p[:, 16:20]
        for rc in range(8):
            k = rc % 4
            dg = d1[:, k] if rc < 4 else d2[:, k]
            Tp = T[:, rc] - np.exp(dg / t) + np.exp(pos[:, k] / t)
            total += float(np.sum(np.log(Tp) - pos[:, k] / t))
    return np.float32(total / N)


def kernel(**inputs) -> np.ndarray:
    nc = _CACHE.get("nc")
    if nc is None:
        nc = _build()
        _CACHE["nc"] = nc
    res = run_bass_kernel_spmd(nc, _prep_in_maps(inputs),
                               core_ids=list(range(NCORES)))
    return _finish_host(res.results)


# revision 20
# speedup vs baseline: 1.0288x; 1.0288x over previous
"""CLIP-style contrastive (NT-Xent) loss on 8 Trainium2 NeuronCores.

Strategy (data-parallel, per sharding hint):
  - Shard the batch (4096) across 8 cores: 512 rows of x_image/x_text each.
  - Each core projects its shard through both towers in TRANSPOSED
    activation layout ([feat_partitions, batch_free]) so every Linear uses
    the stored weight directly as the stationary lhsT (out = lhsT.T @ rhs).
  - ALL per-tower operands (xT chunks, We, Wp1, Wp2) are packed host-side
    into ONE [128, 9472] bf16 DRAM tensor in the exact SBUF image, so the
    whole tower loads with a single DMA instruction (descriptor generation
    on the issuing queue costs ~600ns per dma_start, so the baseline's 73
    small DMAs burned ~44us of sync-queue time).
  - L2-normalize the 128-dim projections on-device: ssq via ones-matmul
    broadcast, Sqrt on ScalarE, fast reciprocal on the DVE (avoids the
    baseline's Ln/Exp activation-table thrash: each table load is 1.3us).
    The Exp table is pre-warmed during the CC-barrier dead window.
  - AllGather the bf16 normalized projections per modality (img AG
    overlaps the txt tower; txt AG overlaps the img-column sim work).
    NOTE (measured): each CC op costs ~11us setup + ~14-18us run, and the
    first collective is gated by a ~43us CC bootstrap/rendezvous barrier
    that starts ~21us into every core's execution regardless of trigger
    time (NEFF launch skew across the 8 cores) -- an early dummy AllGather
    makes things WORSE (CC queue is serial, +9us per extra op).
  - Each core computes its 1024 rows of the global 8192x8192 similarity
    matrix in [128, 2048] PSUM chunks (bf16 matmuls), applies exp(sim/t)
    on ScalarE with fused per-row accumulation (accum_out), giving row
    sums T_r including the self-similarity diagonal.
  - Device returns, per row: T_r, diag_r, pos_r.  Host finishes in fp64:
        T'_r   = T_r - exp(diag_r/t) + exp(pos_r/t)
        loss_r = log(T'_r) - pos_r/t
"""

import os

# Small (256KB/rank) intra-chip AllGathers sit on the critical path; the
# mesh algorithm beats RDH for them (measured 171us -> 164us end-to-end).
os.environ.setdefault("NEURON_RT_DBG_RDH_CC", "0")

import numpy as np
import ml_dtypes

import concourse.bacc as bacc
import concourse.bass as bass
import concourse.mybir as mybir
import concourse.tile as tile
from concourse.bass_utils import run_bass_kernel_spmd
from concourse.dve_ops import RECIPROCAL_APPROX_FAST, RECIP_APPROX_FAST_CONSTS

NCORES = 8
B, DIN, DE, DH, DP = 4096, 1024, 512, 256, 128
S = B // NCORES            # 512: per-core batch shard
ROWS = 2 * S               # 1024 sim rows owned per core (z1 + z2 shard)
N = 2 * B                  # 8192 global rows
TEMP = 0.07
INV_T = 1.0 / TEMP

F32 = mybir.dt.float32
BF16 = mybir.dt.bfloat16
SIM_DT = BF16              # dtype of the similarity matmul operands
PROJ_DT = BF16             # dtype of projection matmul operands
NP_PROJ = ml_dtypes.bfloat16 if PROJ_DT == BF16 else np.float32

# packed tower tensor column offsets (all in PROJ_DT elements)
OXT = 0                    # xT chunks:   8 x [128, 512]
OWE = OXT + (DIN // 128) * S       # We chunks:  8 x [128, 512]
OW1 = OWE + (DIN // 128) * DE      # Wp1 chunks: 4 x [128, 256]
OW2 = OW1 + (DE // 128) * DH       # Wp2 chunks: 2 x [128, 128]
TWC = OW2 + (DH // 128) * DP       # 9472 total columns

# bias_all fp32 [128, 14]: beT(4)|bp1T(2)|bp2T(1) img then txt
BIAS_OFF = {"img": 0, "txt": 7}

# device output layout: [128, 20] = T(8) | pos(4) | diag_img(4) | diag_txt(4)
OUT_COLS = 20

_CACHE: dict = {}


def _build():
    nc = bacc.Bacc("TRN2", target_bir_lowering=False, debug=False,
                   num_devices=NCORES)

    t_in = {
        "tw_img": nc.dram_tensor("tw_img", [128, TWC], PROJ_DT,
                                 kind="ExternalInput"),
        "tw_txt": nc.dram_tensor("tw_txt", [128, TWC], PROJ_DT,
                                 kind="ExternalInput"),
        "bias_all": nc.dram_tensor("bias_all", [128, 14], F32,
                                   kind="ExternalInput"),
    }
    out_t = nc.dram_tensor("parts", [128, OUT_COLS], F32,
                           kind="ExternalOutput")

    with tile.TileContext(nc) as tc:
        _emit(nc, tc, t_in, out_t)
    nc.compile()
    return nc


def _project_normalize(nc, pps, psb, apool, tw, ball, m, ones128):
    """Emit one tower: projections (bf16 matmuls) + fp32 normalize.

    Returns (zn fp32 [128,512], znb SIM_DT [128,512])."""
    add = mybir.AluOpType.add
    mx = mybir.AluOpType.max
    bo = BIAS_OFF[m]

    h = psb.tile([128, (DE // 128) * S], PROJ_DT, tag="h")
    for mm in range(DE // 128):
        ph = pps.tile([128, S], F32, tag="simps")
        for k in range(DIN // 128):
            nc.tensor.matmul(
                ph[:],
                tw[:, OWE + k * DE + 128 * mm: OWE + k * DE + 128 * (mm + 1)],
                tw[:, OXT + k * S: OXT + (k + 1) * S],
                start=(k == 0), stop=(k == DIN // 128 - 1))
        nc.vector.tensor_scalar(
            out=h[:, mm * S:(mm + 1) * S], in0=ph[:],
            scalar1=ball[:, bo + mm: bo + mm + 1], scalar2=None, op0=add)
    g = psb.tile([128, (DH // 128) * S], PROJ_DT, tag="g")
    for mm in range(DH // 128):
        pg = pps.tile([128, S], F32, tag="simps")
        for k in range(DE // 128):
            nc.tensor.matmul(
                pg[:],
                tw[:, OW1 + k * DH + 128 * mm: OW1 + k * DH + 128 * (mm + 1)],
                h[:, k * S:(k + 1) * S],
                start=(k == 0), stop=(k == DE // 128 - 1))
        nc.vector.tensor_scalar(
            out=g[:, mm * S:(mm + 1) * S], in0=pg[:],
            scalar1=ball[:, bo + 4 + mm: bo + 5 + mm], scalar2=0.0,
            op0=add, op1=mx)
    pz = pps.tile([128, S], F32, tag="simps")
    for k in range(DH // 128):
        nc.tensor.matmul(pz[:], tw[:, OW2 + k * DP: OW2 + (k + 1) * DP],
                         g[:, k * S:(k + 1) * S],
                         start=(k == 0), stop=(k == DH // 128 - 1))
    z = psb.tile([128, S], F32, tag=f"z_{m}")
    nc.vector.tensor_scalar(out=z[:], in0=pz[:],
                            scalar1=ball[:, bo + 6: bo + 7],
                            scalar2=None, op0=add)

    # normalize columns (rows of z): inv = 1/sqrt(colsum(z^2)), broadcast
    # to all 128 partitions by the ones128 matmul so the reciprocal runs
    # full-width on the DVE.
    sq = psb.tile([128, S], F32, tag="sq")
    nc.vector.tensor_mul(sq[:], z[:], z[:])
    pssqb = pps.tile([128, S], F32, tag="simps")
    nc.tensor.matmul(pssqb[:], ones128[:], sq[:], start=True, stop=True)
    nrm = psb.tile([128, S], F32, tag="nrm")
    nc.scalar.activation(nrm[:], pssqb[:], mybir.ActivationFunctionType.Sqrt)
    inv = psb.tile([128, S], F32, tag="inv")
    nc.vector._custom_dve(
        RECIPROCAL_APPROX_FAST, out=inv[:], in0=nrm[:],
        s0=RECIP_APPROX_FAST_CONSTS["s0"], s1=RECIP_APPROX_FAST_CONSTS["s1"],
        imm2=RECIP_APPROX_FAST_CONSTS["imm2"])
    zn = apool.tile([128, S], F32, name=f"zn_{m}")
    nc.vector.tensor_mul(zn[:], z[:], inv[:])
    znb = apool.tile([128, S], SIM_DT, name=f"znb_{m}")
    nc.vector.tensor_copy(znb[:], zn[:])
    return zn, znb


def _emit(nc, tc, t_in, out_t):
    Exp = mybir.ActivationFunctionType.Exp
    add = mybir.AluOpType.add

    NCHUNK = 2048                  # columns per PSUM super-chunk (4 banks)
    NTT = N // NCHUNK              # 4
    NRC = ROWS // 128              # 8 row chunks

    with tc.tile_pool(name="const", bufs=1) as cpool, \
         tc.tile_pool(name="wpool", bufs=1) as wpool, \
         tc.tile_pool(name="actpool", bufs=1) as apool, \
         tc.tile_pool(name="projsb", bufs=2) as psb, \
         tc.tile_pool(name="psum", bufs=2, space="PSUM") as pps, \
         tc.tile_pool(name="escp", bufs=2) as escp, \
         tc.tile_pool(name="dram", bufs=1, space="DRAM") as dram:

        # ---- bulk input DMAs: one instruction per packed tensor ----
        tw = {"img": wpool.tile([128, TWC], PROJ_DT, name="tw_img"),
              "txt": wpool.tile([128, TWC], PROJ_DT, name="tw_txt")}
        nc.sync.dma_start(out=tw["img"][:], in_=t_in["tw_img"][:, :])
        nc.scalar.dma_start(out=tw["txt"][:], in_=t_in["tw_txt"][:, :])
        ball = wpool.tile([128, 14], F32, name="ball")
        nc.sync.dma_start(out=ball[:], in_=t_in["bias_all"][:, :])

        ones128 = cpool.tile([128, 128], F32)
        nc.vector.memset(ones128[:], 1.0)

        zn, znb, cc_out = {}, {}, {}
        zf = {"img": apool.tile([128, B], SIM_DT, name="zf_img"),
              "txt": apool.tile([128, B], SIM_DT, name="zf_txt")}
        for m in ("img", "txt"):
            zn[m], znb[m] = _project_normalize(
                nc, pps, psb, apool, tw[m], ball, m, ones128)
            # AllGather this modality right away (img AG overlaps txt tower,
            # txt AG overlaps the img-column sim supersteps).
            cc_in = dram.tile([128, S], SIM_DT, name=f"cc_in_{m}")
            nc.scalar.dma_start(out=cc_in[:, :], in_=znb[m][:])
            cc_o = dram.tile([128 * NCORES, S], SIM_DT, name=f"cc_out_{m}",
                             addr_space="Shared")
            nc.gpsimd.collective_compute(
                "AllGather", mybir.AluOpType.bypass,
                replica_groups=[list(range(NCORES))],
                ins=[cc_in[:]], outs=[cc_o[:]])
            cc_out[m] = cc_o
            # one rearranged DMA gathers all 8 rank slabs into SBUF
            nc.sync.dma_start(
                out=zf[m][:].rearrange("p (j c) -> p j c", j=NCORES),
                in_=cc_o[:].rearrange("(j p) c -> p j c", p=128))

        # Warm the Exp activation table on ScalarE now (pre-barrier dead
        # window) so the first sim activation skips the 1.3us table load.
        warm = psb.tile([1, 16], F32, tag="rowsb")
        nc.scalar.activation(warm[:], ones128[0:1, 0:16], Exp)

        # pos / self-diag rows ([1, 512] each) -> [128, 12] via DRAM bounce.
        # Products use znb (the bf16 values the sim matmul actually sees) so
        # the host's diag/pos correction cancels the in-matrix terms exactly.
        rows_d = dram.tile([3, S], F32)
        for r, (a, b) in enumerate((("img", "txt"), ("img", "img"),
                                    ("txt", "txt"))):
            prod = psb.tile([128, S], F32, tag="sq")
            nc.vector.tensor_mul(prod[:], znb[a][:], znb[b][:])
            pr = pps.tile([1, S], F32, tag="simps")
            nc.tensor.matmul(pr[:], ones128[:, 0:1], prod[:], start=True,
                             stop=True)
            row_sb = psb.tile([1, S], F32, tag="rowsb")
            nc.vector.tensor_copy(row_sb[:], pr[:])
            nc.scalar.dma_start(out=rows_d[r:r + 1, :], in_=row_sb[:])

        pdT = apool.tile([128, 12], F32)   # pos | diag_img | diag_txt
        nc.scalar.dma_start(
            out=pdT[:],
            in_=rows_d[:, :].rearrange("r (c p) -> p (r c)", p=128))

        # ---- main loop: sim rows + exp + fused row sums ----
        # image columns (ready after AG1) run before text columns.
        stats = apool.tile([128, NRC * NTT], F32)
        for tt in range(NTT):
            src = zf["img"] if tt < NTT // 2 else zf["txt"]
            coff = (tt % (NTT // 2)) * NCHUNK
            for rc in range(NRC):
                if rc < 4:
                    lhs = znb["img"][:, 128 * rc:128 * (rc + 1)]
                else:
                    lhs = znb["txt"][:, 128 * (rc - 4):128 * (rc - 3)]
                ps = pps.tile([128, NCHUNK], F32, tag="simps")
                for q in range(NCHUNK // 512):
                    nc.tensor.matmul(
                        ps[:, 512 * q:512 * (q + 1)], lhs,
                        src[:, coff + 512 * q: coff + 512 * (q + 1)],
                        start=True, stop=True)
                esc = escp.tile([128, NCHUNK], SIM_DT, tag="esc")
                nc.scalar.activation(
                    esc[:], ps[:], Exp, scale=INV_T,
                    accum_out=stats[:, NTT * rc + tt: NTT * rc + tt + 1])

        # ---- gather outputs: T (8) | pos(4) | diag1(4) | diag2(4) ----
        outv = apool.tile([128, OUT_COLS], F32)
        nc.vector.tensor_reduce(
            out=outv[:, 0:NRC],
            in_=stats[:].rearrange("p (r t) -> p r t", t=NTT),
            axis=mybir.AxisListType.X, op=add)
        nc.vector.tensor_copy(outv[:, NRC:NRC + 12], pdT[:])
        nc.sync.dma_start(out=out_t[:, :], in_=outv[:])


def _chunkT(a):
    """[R, C] (R = 128*k) -> [128, k*C] with chunk k at cols [k*C, (k+1)*C)."""
    r, c = a.shape
    return a.reshape(r // 128, 128, c).transpose(1, 0, 2).reshape(128, -1)


def _prep_in_maps(inputs):
    wpart, biases = {}, {}
    for m in ("img", "txt"):
        wpart[m] = np.concatenate(
            [_chunkT(np.asarray(inputs[f"We_{m}"], np.float32)),
             _chunkT(np.asarray(inputs[f"Wp1_{m}"], np.float32)),
             _chunkT(np.asarray(inputs[f"Wp2_{m}"], np.float32))],
            axis=1).astype(NP_PROJ)
        biases[m] = np.concatenate(
            [np.asarray(inputs[f"be_{m}"], np.float32).reshape(DE // 128, 128).T,
             np.asarray(inputs[f"bp1_{m}"], np.float32).reshape(DH // 128, 128).T,
             np.asarray(inputs[f"bp2_{m}"], np.float32).reshape(DP // 128, 128).T],
            axis=1)
    bias_all = np.ascontiguousarray(
        np.concatenate([biases["img"], biases["txt"]], axis=1))
    x = {"img": np.asarray(inputs["x_image"], np.float32),
         "txt": np.asarray(inputs["x_text"], np.float32)}
    in_maps = []
    for c in range(NCORES):
        mp = {"bias_all": bias_all}
        for m in ("img", "txt"):
            xt = _chunkT(x[m][c * S:(c + 1) * S].T).astype(NP_PROJ)
            mp[f"tw_{m}"] = np.ascontiguousarray(
                np.concatenate([xt, wpart[m]], axis=1))
        in_maps.append(mp)
    return in_maps


def _finish_host(results):
    """Host-side fp64 finish: combine per-core T/pos/diag into the loss."""
    total = 0.0
    t = TEMP
    for c in range(NCORES):
        p = np.asarray(results[c]["parts"], np.float64)
        T = p[:, 0:8]           # [128, rc]
        pos = p[:, 8:12]        # [128, k]  (k = batch chunk within shard)
        d1 = p[:, 12:16]
        d2 = p[:, 16:20]
        for rc in range(8):
            k = rc % 4
            dg = d1[:, k] if rc < 4 else d2[:, k]
            Tp = T[:, rc] - np.exp(dg / t) + np.exp(pos[:, k] / t)
            total += float(np.sum(np.log(Tp) - pos[:, k] / t))
    return np.float32(total / N)


def kernel(**inputs) -> np.ndarray:
    nc = _CACHE.get("nc")
    if nc is None:
        nc = _build()
        _CACHE["nc"] = nc
    res = run_bass_kernel_spmd(nc, _prep_in_maps(inputs),
                               core_ids=list(range(NCORES)))
    return _finish_host(res.results)


# revision 22
# speedup vs baseline: 1.0989x; 1.0681x over previous
"""CLIP-style contrastive (NT-Xent) loss on 8 Trainium2 NeuronCores.

Strategy (data-parallel, per sharding hint):
  - Shard the batch (4096) across 8 cores: 512 rows of x_image/x_text each.
  - Each core projects its shard through both towers in TRANSPOSED
    activation layout ([feat_partitions, batch_free]) so every Linear uses
    the stored weight directly as the stationary lhsT (out = lhsT.T @ rhs).
  - ALL per-tower operands (xT chunks, We, Wp1, Wp2) are packed host-side
    into ONE [128, 9472] bf16 DRAM tensor in the exact SBUF image, so the
    whole tower loads with a single DMA instruction (descriptor generation
    on the issuing queue costs ~600ns per dma_start, so the baseline's 73
    small DMAs burned ~44us of sync-queue time).
  - L2-normalize the 128-dim projections on-device: ssq via ones-matmul
    broadcast, Sqrt on ScalarE, fast reciprocal on the DVE (avoids the
    baseline's Ln/Exp activation-table thrash: each table load is 1.3us).
    The Exp table is pre-warmed during the CC-barrier dead window.
  - AllGather the bf16 normalized projections per modality (img AG
    overlaps the txt tower; txt AG overlaps the img-column sim work).
    NOTE (measured): each CC op costs ~11us setup + ~14-18us run, and the
    first collective is gated by a ~43us CC bootstrap/rendezvous barrier
    that starts ~21us into every core's execution regardless of trigger
    time (NEFF launch skew across the 8 cores) -- an early dummy AllGather
    makes things WORSE (CC queue is serial, +9us per extra op).
  - Each core computes its 1024 rows of the global 8192x8192 similarity
    matrix in [128, 2048] PSUM chunks (bf16 matmuls), applies exp(sim/t)
    on ScalarE with fused per-row accumulation (accum_out), giving row
    sums T_r including the self-similarity diagonal.
  - Device returns, per row: T_r, diag_r, pos_r.  Host finishes in fp64:
        T'_r   = T_r - exp(diag_r/t) + exp(pos_r/t)
        loss_r = log(T'_r) - pos_r/t
"""

import os

# Small (256KB/rank) intra-chip AllGathers sit on the critical path; the
# mesh algorithm beats RDH for them (measured 171us -> 164us end-to-end).
os.environ.setdefault("NEURON_RT_DBG_RDH_CC", "0")

import numpy as np
import ml_dtypes

import concourse.bacc as bacc
import concourse.bass as bass
import concourse.mybir as mybir
import concourse.tile as tile
from concourse.bass_utils import run_bass_kernel_spmd
from concourse.dve_ops import RECIPROCAL_APPROX_FAST, RECIP_APPROX_FAST_CONSTS

NCORES = 8
B, DIN, DE, DH, DP = 4096, 1024, 512, 256, 128
S = B // NCORES            # 512: per-core batch shard
ROWS = 2 * S               # 1024 sim rows owned per core (z1 + z2 shard)
N = 2 * B                  # 8192 global rows
TEMP = 0.07
INV_T = 1.0 / TEMP

F32 = mybir.dt.float32
BF16 = mybir.dt.bfloat16
SIM_DT = mybir.dt.float8e4  # sim matmul operands (halves AG payload)
PROJ_DT = BF16             # dtype of projection matmul operands
NP_PROJ = ml_dtypes.bfloat16 if PROJ_DT == BF16 else np.float32

# packed tower tensor column offsets (all in PROJ_DT elements)
OXT = 0                    # xT chunks:   8 x [128, 512]
OWE = OXT + (DIN // 128) * S       # We chunks:  8 x [128, 512]
OW1 = OWE + (DIN // 128) * DE      # Wp1 chunks: 4 x [128, 256]
OW2 = OW1 + (DE // 128) * DH       # Wp2 chunks: 2 x [128, 128]
TWC = OW2 + (DH // 128) * DP       # 9472 total columns

# bias_all fp32 [128, 14]: beT(4)|bp1T(2)|bp2T(1) img then txt
BIAS_OFF = {"img": 0, "txt": 7}

# device output layout: [128, 20] = T(8) | pos(4) | diag_img(4) | diag_txt(4)
OUT_COLS = 20

_CACHE: dict = {}


def _build():
    nc = bacc.Bacc("TRN2", target_bir_lowering=False, debug=False,
                   num_devices=NCORES)

    t_in = {
        "tw_img": nc.dram_tensor("tw_img", [128, TWC], PROJ_DT,
                                 kind="ExternalInput"),
        "tw_txt": nc.dram_tensor("tw_txt", [128, TWC], PROJ_DT,
                                 kind="ExternalInput"),
        "bias_all": nc.dram_tensor("bias_all", [128, 14], F32,
                                   kind="ExternalInput"),
    }
    out_t = nc.dram_tensor("parts", [128, OUT_COLS], F32,
                           kind="ExternalOutput")

    with tile.TileContext(nc) as tc:
        _emit(nc, tc, t_in, out_t)
    nc.compile()
    return nc


def _project_normalize(nc, pps, psb, apool, tw, ball, m, ones128):
    """Emit one tower: projections (bf16 matmuls) + fp32 normalize.

    Returns (zn fp32 [128,512], znb SIM_DT [128,512])."""
    add = mybir.AluOpType.add
    mx = mybir.AluOpType.max
    bo = BIAS_OFF[m]

    h = psb.tile([128, (DE // 128) * S], PROJ_DT, tag="h")
    for mm in range(DE // 128):
        ph = pps.tile([128, S], F32, tag="simps")
        for k in range(DIN // 128):
            nc.tensor.matmul(
                ph[:],
                tw[:, OWE + k * DE + 128 * mm: OWE + k * DE + 128 * (mm + 1)],
                tw[:, OXT + k * S: OXT + (k + 1) * S],
                start=(k == 0), stop=(k == DIN // 128 - 1))
        nc.vector.tensor_scalar(
            out=h[:, mm * S:(mm + 1) * S], in0=ph[:],
            scalar1=ball[:, bo + mm: bo + mm + 1], scalar2=None, op0=add)
    g = psb.tile([128, (DH // 128) * S], PROJ_DT, tag="g")
    for mm in range(DH // 128):
        pg = pps.tile([128, S], F32, tag="simps")
        for k in range(DE // 128):
            nc.tensor.matmul(
                pg[:],
                tw[:, OW1 + k * DH + 128 * mm: OW1 + k * DH + 128 * (mm + 1)],
                h[:, k * S:(k + 1) * S],
                start=(k == 0), stop=(k == DE // 128 - 1))
        nc.vector.tensor_scalar(
            out=g[:, mm * S:(mm + 1) * S], in0=pg[:],
            scalar1=ball[:, bo + 4 + mm: bo + 5 + mm], scalar2=0.0,
            op0=add, op1=mx)
    pz = pps.tile([128, S], F32, tag="simps")
    for k in range(DH // 128):
        nc.tensor.matmul(pz[:], tw[:, OW2 + k * DP: OW2 + (k + 1) * DP],
                         g[:, k * S:(k + 1) * S],
                         start=(k == 0), stop=(k == DH // 128 - 1))
    z = psb.tile([128, S], F32, tag=f"z_{m}")
    nc.vector.tensor_scalar(out=z[:], in0=pz[:],
                            scalar1=ball[:, bo + 6: bo + 7],
                            scalar2=None, op0=add)

    # normalize columns (rows of z): inv = 1/sqrt(colsum(z^2)), broadcast
    # to all 128 partitions by the ones128 matmul so the reciprocal runs
    # full-width on the DVE.
    sq = psb.tile([128, S], F32, tag="sq")
    nc.vector.tensor_mul(sq[:], z[:], z[:])
    pssqb = pps.tile([128, S], F32, tag="simps")
    nc.tensor.matmul(pssqb[:], ones128[:], sq[:], start=True, stop=True)
    nrm = psb.tile([128, S], F32, tag="nrm")
    nc.scalar.activation(nrm[:], pssqb[:], mybir.ActivationFunctionType.Sqrt)
    inv = psb.tile([128, S], F32, tag="inv")
    nc.vector._custom_dve(
        RECIPROCAL_APPROX_FAST, out=inv[:], in0=nrm[:],
        s0=RECIP_APPROX_FAST_CONSTS["s0"], s1=RECIP_APPROX_FAST_CONSTS["s1"],
        imm2=RECIP_APPROX_FAST_CONSTS["imm2"])
    zn = apool.tile([128, S], F32, name=f"zn_{m}")
    nc.vector.tensor_mul(zn[:], z[:], inv[:])
    znb = apool.tile([128, S], SIM_DT, name=f"znb_{m}")
    nc.vector.tensor_copy(znb[:], zn[:])
    return zn, znb


def _emit(nc, tc, t_in, out_t):
    Exp = mybir.ActivationFunctionType.Exp
    add = mybir.AluOpType.add

    NCHUNK = 2048                  # columns per PSUM super-chunk (4 banks)
    NTT = N // NCHUNK              # 4
    NRC = ROWS // 128              # 8 row chunks

    with tc.tile_pool(name="const", bufs=1) as cpool, \
         tc.tile_pool(name="wpool", bufs=1) as wpool, \
         tc.tile_pool(name="actpool", bufs=1) as apool, \
         tc.tile_pool(name="projsb", bufs=2) as psb, \
         tc.tile_pool(name="psum", bufs=2, space="PSUM") as pps, \
         tc.tile_pool(name="escp", bufs=2) as escp, \
         tc.tile_pool(name="dram", bufs=1, space="DRAM") as dram:

        # ---- bulk input DMAs: one instruction per packed tensor ----
        tw = {"img": wpool.tile([128, TWC], PROJ_DT, name="tw_img"),
              "txt": wpool.tile([128, TWC], PROJ_DT, name="tw_txt")}
        nc.sync.dma_start(out=tw["img"][:], in_=t_in["tw_img"][:, :])
        nc.scalar.dma_start(out=tw["txt"][:], in_=t_in["tw_txt"][:, :])
        ball = wpool.tile([128, 14], F32, name="ball")
        nc.sync.dma_start(out=ball[:], in_=t_in["bias_all"][:, :])

        ones128 = cpool.tile([128, 128], F32)
        nc.vector.memset(ones128[:], 1.0)

        zn, znb, cc_out = {}, {}, {}
        zf = {"img": apool.tile([128, B], SIM_DT, name="zf_img"),
              "txt": apool.tile([128, B], SIM_DT, name="zf_txt")}
        for m in ("img", "txt"):
            zn[m], znb[m] = _project_normalize(
                nc, pps, psb, apool, tw[m], ball, m, ones128)
            # AllGather this modality right away (img AG overlaps txt tower,
            # txt AG overlaps the img-column sim supersteps).
            cc_in = dram.tile([128, S], SIM_DT, name=f"cc_in_{m}")
            nc.scalar.dma_start(out=cc_in[:, :], in_=znb[m][:])
            cc_o = dram.tile([128 * NCORES, S], SIM_DT, name=f"cc_out_{m}",
                             addr_space="Shared")
            nc.gpsimd.collective_compute(
                "AllGather", mybir.AluOpType.bypass,
                replica_groups=[list(range(NCORES))],
                ins=[cc_in[:]], outs=[cc_o[:]])
            cc_out[m] = cc_o
            # one rearranged DMA gathers all 8 rank slabs into SBUF
            nc.sync.dma_start(
                out=zf[m][:].rearrange("p (j c) -> p j c", j=NCORES),
                in_=cc_o[:].rearrange("(j p) c -> p j c", p=128))

        # Warm the Exp activation table on ScalarE now (pre-barrier dead
        # window) so the first sim activation skips the 1.3us table load.
        warm = psb.tile([1, 16], F32, tag="rowsb")
        nc.scalar.activation(warm[:], ones128[0:1, 0:16], Exp)

        # pos / self-diag rows ([1, 512] each) -> [128, 12] via DRAM bounce.
        # Products use znb (the bf16 values the sim matmul actually sees) so
        # the host's diag/pos correction cancels the in-matrix terms exactly.
        rows_d = dram.tile([3, S], F32)
        for r, (a, b) in enumerate((("img", "txt"), ("img", "img"),
                                    ("txt", "txt"))):
            prod = psb.tile([128, S], F32, tag="sq")
            nc.vector.tensor_mul(prod[:], znb[a][:], znb[b][:])
            pr = pps.tile([1, S], F32, tag="simps")
            nc.tensor.matmul(pr[:], ones128[:, 0:1], prod[:], start=True,
                             stop=True)
            row_sb = psb.tile([1, S], F32, tag="rowsb")
            nc.vector.tensor_copy(row_sb[:], pr[:])
            nc.scalar.dma_start(out=rows_d[r:r + 1, :], in_=row_sb[:])

        pdT = apool.tile([128, 12], F32)   # pos | diag_img | diag_txt
        nc.scalar.dma_start(
            out=pdT[:],
            in_=rows_d[:, :].rearrange("r (c p) -> p (r c)", p=128))

        # ---- main loop: sim rows + exp + fused row sums ----
        # image columns (ready after AG1) run before text columns.
        stats = apool.tile([128, NRC * NTT], F32)
        for tt in range(NTT):
            src = zf["img"] if tt < NTT // 2 else zf["txt"]
            coff = (tt % (NTT // 2)) * NCHUNK
            for rc in range(NRC):
                if rc < 4:
                    lhs = znb["img"][:, 128 * rc:128 * (rc + 1)]
                else:
                    lhs = znb["txt"][:, 128 * (rc - 4):128 * (rc - 3)]
                ps = pps.tile([128, NCHUNK], F32, tag="simps")
                for q in range(NCHUNK // 512):
                    nc.tensor.matmul(
                        ps[:, 512 * q:512 * (q + 1)], lhs,
                        src[:, coff + 512 * q: coff + 512 * (q + 1)],
                        start=True, stop=True)
                esc = escp.tile([128, NCHUNK], BF16, tag="esc")
                nc.scalar.activation(
                    esc[:], ps[:], Exp, scale=INV_T,
                    accum_out=stats[:, NTT * rc + tt: NTT * rc + tt + 1])

        # ---- gather outputs: T (8) | pos(4) | diag1(4) | diag2(4) ----
        outv = apool.tile([128, OUT_COLS], F32)
        nc.vector.tensor_reduce(
            out=outv[:, 0:NRC],
            in_=stats[:].rearrange("p (r t) -> p r t", t=NTT),
            axis=mybir.AxisListType.X, op=add)
        nc.vector.tensor_copy(outv[:, NRC:NRC + 12], pdT[:])
        nc.sync.dma_start(out=out_t[:, :], in_=outv[:])


def _chunkT(a):
    """[R, C] (R = 128*k) -> [128, k*C] with chunk k at cols [k*C, (k+1)*C)."""
    r, c = a.shape
    return a.reshape(r // 128, 128, c).transpose(1, 0, 2).reshape(128, -1)


def _prep_in_maps(inputs):
    wpart, biases = {}, {}
    for m in ("img", "txt"):
        wpart[m] = np.concatenate(
            [_chunkT(np.asarray(inputs[f"We_{m}"], np.float32)),
             _chunkT(np.asarray(inputs[f"Wp1_{m}"], np.float32)),
             _chunkT(np.asarray(inputs[f"Wp2_{m}"], np.float32))],
            axis=1).astype(NP_PROJ)
        biases[m] = np.concatenate(
            [np.asarray(inputs[f"be_{m}"], np.float32).reshape(DE // 128, 128).T,
             np.asarray(inputs[f"bp1_{m}"], np.float32).reshape(DH // 128, 128).T,
             np.asarray(inputs[f"bp2_{m}"], np.float32).reshape(DP // 128, 128).T],
            axis=1)
    bias_all = np.ascontiguousarray(
        np.concatenate([biases["img"], biases["txt"]], axis=1))
    x = {"img": np.asarray(inputs["x_image"], np.float32),
         "txt": np.asarray(inputs["x_text"], np.float32)}
    in_maps = []
    for c in range(NCORES):
        mp = {"bias_all": bias_all}
        for m in ("img", "txt"):
            xt = _chunkT(x[m][c * S:(c + 1) * S].T).astype(NP_PROJ)
            mp[f"tw_{m}"] = np.ascontiguousarray(
                np.concatenate([xt, wpart[m]], axis=1))
        in_maps.append(mp)
    return in_maps


def _finish_host(results):
    """Host-side fp64 finish: combine per-core T/pos/diag into the loss."""
    total = 0.0
    t = TEMP
    for c in range(NCORES):
        p = np.asarray(results[c]["parts"], np.float64)
        T = p[:, 0:8]           # [128, rc]
        pos = p[:, 8:12]        # [128, k]  (k = batch chunk within shard)
        d1 = p[:, 12:16]
        d2 = p[:, 16:20]
        for rc in range(8):
            k = rc % 4
            dg = d1[:, k] if rc < 4 else d2[:, k]
            Tp = T[:, rc] - np.exp(dg / t) + np.exp(pos[:, k] / t)
            total += float(np.sum(np.log(Tp) - pos[:, k] / t))
    return np.float32(total / N)


def kernel(**inputs) -> np.ndarray:
    nc = _CACHE.get("nc")
    if nc is None:
        nc = _build()
        _CACHE["nc"] = nc
    res = run_bass_kernel_spmd(nc, _prep_in_maps(inputs),
                               core_ids=list(range(NCORES)))
    return _finish_host(res.results)
